# revision 5
# baseline (speedup 1.0000x reference)
"""Trainium2 Bass kernel: multi-head attention (dense transformer block).

Computation (per batch b):
    Q = x @ Wq + bq ; K = x @ Wk + bk ; V = x @ Wv + bv        (per head)
    P = exp((Q @ K^T) / sqrt(Dh))                   (no max-subtraction needed:
                                                     scores are O(1) by construction)
    out = sum_h (P @ V / rowsum(P)) @ Wd[h] + bd

Sharding (data + tensor parallel): 8 cores; core c handles batch b = c // 4
and the 4 heads starting at 4*(c % 4). Each core computes a partial [L, D]
output; the host sums the 4 partials per batch and adds bd.

v2 design notes (per-core):
  - The kernel is a PE/ACT "ridge": matmul stream floor ~143us, exp on the
    scalar(ACT) engine ~128-146us. ScalarE therefore runs NOTHING but the
    128 exp instructions; every drain/cast/copy lives on DVE or GpSimd.
  - All input DMAs are SWDGE (gpsimd) casting fp32->bf16 in flight: no
    staging tiles, no cast instructions. Issue order = priority: biases,
    Wk/Wq(pair0), x tiles (with pair1 / Wv / Wd slotted between), so the
    first K-projection starts ~8us in and the first exp ~11us in.
  - x^T built on-chip via PE transposes (128x128, identity rhs), DVE drain.
  - Per pair: K^T then Q^T chunks (bf16, dual-head 64-row stacking), scores
    S^T = K^T.T @ Q^T land softmax-axis-on-partitions; two heads run on
    independent 64-row PE tiles concurrently. exp on ScalarE (scale=1/8
    fused), 1024 wide, PSUM->SBUF bf16.
  - V for BOTH pairs in one pass ([l',d] layout, N=256 matmuls); drain is a
    single strided DVE add that also applies bv and interleaves with the
    ones-columns used for the denominator trick.
  - attend: O^T = [V_h | ones].T @ P^T; PSUM rows 64..127 = rowsum(P),
    broadcast for free. Normalize: DVE copy + reciprocal_approx_fast
    (~5x faster than the iterative divide) + GpSimd multiply.
  - out-proj is interleaved: as each pair-1 512-chunk normalizes, the
    covered l-tiles' Y = O^T.T @ Wd fire, DVE-drain, and DMA out, so the
    tail is one chunk deep instead of the whole projection.
All matmul operands are bf16 (fp32 accumulation in PSUM).
"""

import os
import sys
from contextlib import ExitStack

import numpy as np

for _p in ("/opt/trn_rl_repo", "/root/.axon_site/_ro/trn_rl_repo"):
    if os.path.isdir(_p) and _p not in sys.path:
        sys.path.append(_p)

import concourse.bass as bass
import concourse.tile as tile
from concourse import bacc, mybir
from concourse.bass import ds, ts
from concourse.bass_utils import run_bass_kernel_spmd
from concourse.masks import make_identity

F32 = mybir.dt.float32
BF16 = mybir.dt.bfloat16

# Problem sizes (hardcoded per contract).
DMODEL, HEADS, DHEAD = 1024, 16, 64
B, L = 2, 2048
NCORES = 8
H_PER_CORE = B * HEADS // NCORES          # 4 heads per core
NPAIR = H_PER_CORE // 2                   # head pairs per core
P = 128                                   # partitions
KT = DMODEL // P                          # 8 k-tiles over dmodel
NLT = L // P                              # 16 l-tiles
LCH = 512                                 # matmul free-dim chunk (one psum bank)
ECH = 1024                                # exp chunk (2 psum banks)
NEC = L // ECH                            # 2 exp chunks
MCH = 512                                 # m-chunk for out-proj
NMC = DMODEL // MCH
PT_BUFS = 40                              # score-tile ring (SBUF budget bound)


def build_nc():
    """Build the SPMD Bass program for one core."""
    nc = bacc.Bacc("TRN2", target_bir_lowering=False, debug=False,
                   num_devices=NCORES)

    x_d = nc.dram_tensor("x", [L, DMODEL], F32, kind="ExternalInput").ap()
    wq_d = nc.dram_tensor("wq", [DMODEL, H_PER_CORE * DHEAD], F32, kind="ExternalInput").ap()
    wk_d = nc.dram_tensor("wk", [DMODEL, H_PER_CORE * DHEAD], F32, kind="ExternalInput").ap()
    wv_d = nc.dram_tensor("wv", [DMODEL, H_PER_CORE * DHEAD], F32, kind="ExternalInput").ap()
    wd_d = nc.dram_tensor("wd", [H_PER_CORE * DHEAD, DMODEL], F32, kind="ExternalInput").ap()
    bq_d = nc.dram_tensor("bq", [H_PER_CORE * DHEAD], F32, kind="ExternalInput").ap()
    bk_d = nc.dram_tensor("bk", [H_PER_CORE * DHEAD], F32, kind="ExternalInput").ap()
    bv_d = nc.dram_tensor("bv", [H_PER_CORE * DHEAD], F32, kind="ExternalInput").ap()
    y_d = nc.dram_tensor("y", [L, DMODEL], F32, kind="ExternalOutput").ap()

    with ExitStack() as ctx:
        tc = ctx.enter_context(tile.TileContext(nc))
        _body(nc, tc, ctx, x_d, wq_d, wk_d, wv_d, wd_d, bq_d, bk_d, bv_d, y_d)
    nc.compile()
    return nc


def _body(nc, tc, ctx, x_d, wq_d, wk_d, wv_d, wd_d, bq_d, bk_d, bv_d, y_d):
    const = ctx.enter_context(tc.tile_pool(name="const", bufs=1))
    sb = ctx.enter_context(tc.tile_pool(name="sb", bufs=1))
    psum = ctx.enter_context(tc.tile_pool(name="psum", bufs=1, space="PSUM"))

    ident = const.tile([P, P], BF16)
    make_identity(nc, ident)

    # ---- SWDGE input DMAs; issue order is the priority order ----
    # biases: [P, {q,k}, pair] per-partition scalars for the q/k drains
    bias_sb = const.tile([P, 2, NPAIR], F32)
    for i, b_d in enumerate((bq_d, bk_d)):
        for p in range(NPAIR):
            nc.gpsimd.dma_start(bias_sb[:, i, p:p + 1],
                                b_d.rearrange("(a p) -> a p", p=P)[p:p + 1, :]
                                .rearrange("a p -> p a"))
    # bv replicated across partitions, (pair, head, d) order = bv flat order
    bv_rep = const.tile([P, NPAIR * P], F32)
    nc.gpsimd.dma_start(bv_rep, bass.AP(tensor=bv_d.tensor, offset=0,
                                        ap=[[0, P], [1, NPAIR * P]]))

    w_sb = const.tile([P, NPAIR, 3, KT, P], BF16)   # [k, pair, {q,k,v}, kt, cols]
    wd_sb = const.tile([P, NPAIR, DMODEL], BF16)

    def w_dma(p, i, w_d):
        nc.gpsimd.dma_start(
            w_sb[:, p, i],
            w_d.rearrange("(kt k) m -> k kt m", k=P)[:, :, ds(p * P, P)])

    w_dma(0, 1, wk_d)   # pair-0 K first: first projection to run
    w_dma(0, 0, wq_d)

    # shared [128,1024] psum slots for scores & transposes
    def sc_tile(shape=None, dt=F32):
        return psum.tile(shape or [P, ECH], dt, tag="sctr", bufs=2, name="sctr")

    # ---- phase 0: x cast-DMA (fp32->bf16 in flight) + PE transposes ----
    xt = sb.tile([P, KT, L], BF16)
    for lt in range(NLT):
        xb = sb.tile([P, DMODEL], BF16, tag="xb", bufs=3)
        nc.gpsimd.dma_start(xb, x_d[ds(lt * P, P), :])
        if lt == 3:
            w_dma(1, 1, wk_d)
            w_dma(1, 0, wq_d)
        elif lt == 7:
            w_dma(0, 2, wv_d)
            w_dma(1, 2, wv_d)
        elif lt == 11:
            nc.gpsimd.dma_start(wd_sb, wd_d.rearrange("(pp k) m -> k pp m", k=P))
        tp = sc_tile([P, KT, P], BF16)
        for kt in range(KT):
            nc.tensor.transpose(tp[:, kt], xb[:, ds(kt * P, P)], ident)
        nc.vector.tensor_copy(xt[:, :, ds(lt * P, P)], tp)

    o_norm = sb.tile([P, NPAIR, L], BF16)
    # V for both pairs in [l', (pair,head)|{v,ones}] interleaved layout
    vt = sb.tile([P, NLT, 2 * NPAIR, 2, DHEAD], BF16)

    # ---- per pair: K -> Q(chunk0) -> scores/exp/attend blocks ----
    def qkv_proj(dst, p, i, lcs):
        for lc in lcs:
            ps = psum.tile([P, LCH], F32, tag="qkvp", bufs=2, name="qkvps")
            for kt in range(KT):
                nc.tensor.matmul(
                    ps, lhsT=w_sb[:, p, i, kt],
                    rhs=xt[:, kt, ds(lc * LCH, LCH)],
                    start=(kt == 0), stop=(kt == KT - 1))
            nc.vector.tensor_scalar_add(
                dst[:, ds(lc * LCH, LCH)], ps, bias_sb[:, i, p:p + 1])

    for p in range(NPAIR):
        qT = sb.tile([P, L], BF16, tag="qkv0", bufs=NPAIR)
        kT_sb = sb.tile([P, L], BF16, tag="qkv1", bufs=NPAIR)
        qkv_proj(kT_sb, p, 1, range(L // LCH))
        qkv_proj(qT, p, 0, range(ECH // LCH))

        for ec in range(NEC):
            if ec > 0:
                qkv_proj(qT, p, 0, range(ec * ECH // LCH, (ec + 1) * ECH // LCH))
            pt_tiles = [[None] * NLT, [None] * NLT]
            for lt in range(NLT):
                for h in range(2):
                    sp = sc_tile()
                    for sub in range(ECH // LCH):
                        nc.tensor.matmul(
                            sp[:, ds(sub * LCH, LCH)],
                            lhsT=kT_sb[ds(64 * h, 64), ds(lt * P, P)],
                            rhs=qT[ds(64 * h, 64),
                                   ds(ec * ECH + sub * LCH, LCH)],
                            start=True, stop=True)
                    pt = sb.tile([P, ECH], BF16, tag="pt", bufs=PT_BUFS)
                    nc.scalar.activation(
                        pt, sp, func=mybir.ActivationFunctionType.Exp,
                        scale=1.0 / np.sqrt(DHEAD))
                    pt_tiles[h][lt] = pt

            if p == 0 and ec == 0:
                # V (both pairs) in [l', d] layout, emitted after the first
                # scores so the exp pipeline starts as early as possible
                nc.vector.memset(vt[:, :, :, 1, :], 1.0)
                for lt in range(NLT):
                    vp = psum.tile([P, 2 * NPAIR * DHEAD], F32,
                                   tag="qkvp", bufs=2, name="vp")
                    for kt in range(KT):
                        nc.tensor.matmul(
                            vp, lhsT=xt[:, kt, ds(lt * P, P)],
                            rhs=w_sb[:, :, 2, kt, :],
                            start=(kt == 0), stop=(kt == KT - 1))
                    nc.vector.tensor_add(
                        vt[:, lt, :, 0, :],
                        vp.rearrange("a (g d) -> a g d", d=DHEAD),
                        bv_rep.rearrange("a (g d) -> a g d", d=DHEAD))

            for sub in range(ECH // LCH):
                lc = ec * ECH + sub * LCH
                for h in range(2):
                    op = psum.tile([P, LCH], F32, tag="op", bufs=2)
                    for lt in range(NLT):
                        nc.tensor.matmul(
                            op, lhsT=vt[:, lt, 2 * p + h],
                            rhs=pt_tiles[h][lt][:, ds(sub * LCH, LCH)],
                            start=(lt == 0), stop=(lt == NLT - 1))
                    # rows 64..127 are the denominator, already broadcast
                    os_sb = sb.tile([P, LCH], F32, tag="os", bufs=3)
                    nc.vector.tensor_copy(os_sb, op)
                    rs = sb.tile([DHEAD, LCH], F32, tag="rs", bufs=2)
                    nc.vector.reciprocal(rs, os_sb[DHEAD:P, :])
                    nc.gpsimd.tensor_mul(
                        o_norm[ds(64 * h, 64), p, ds(lc, LCH)],
                        os_sb[0:DHEAD, :], rs)

                if p == NPAIR - 1:
                    # out-proj for the l-tiles this chunk completed
                    for lt in range(lc // P, lc // P + LCH // P):
                        for mc in range(NMC):
                            yp = psum.tile([P, MCH], F32, tag="qkvp",
                                           bufs=2, name="yp")
                            for pp in range(NPAIR):
                                nc.tensor.matmul(
                                    yp, lhsT=o_norm[:, pp, ds(lt * P, P)],
                                    rhs=wd_sb[:, pp, ds(mc * MCH, MCH)],
                                    start=(pp == 0), stop=(pp == NPAIR - 1))
                            ys = sb.tile([P, MCH], F32, tag="ys", bufs=3)
                            nc.vector.tensor_copy(ys, yp)
                            nc.sync.dma_start(
                                y_d[ds(lt * P, P), ds(mc * MCH, MCH)], ys)


_NC_CACHE = {}


def _get_nc():
    if "nc" not in _NC_CACHE:
        _NC_CACHE["nc"] = build_nc()
    return _NC_CACHE["nc"]


def shard_inputs(x, Wq, bq, Wk, bk, Wv, bv, Wd, bd):
    """Build the 8 per-core input maps."""
    in_maps = []
    for c in range(NCORES):
        b = c // (NCORES // B)
        h0 = (c % (NCORES // B)) * H_PER_CORE
        hs = slice(h0, h0 + H_PER_CORE)
        in_maps.append({
            "x": np.ascontiguousarray(np.asarray(x[b], np.float32)),
            "wq": np.ascontiguousarray(np.asarray(Wq[:, hs, :], np.float32).reshape(DMODEL, -1)),
            "wk": np.ascontiguousarray(np.asarray(Wk[:, hs, :], np.float32).reshape(DMODEL, -1)),
            "wv": np.ascontiguousarray(np.asarray(Wv[:, hs, :], np.float32).reshape(DMODEL, -1)),
            "wd": np.ascontiguousarray(np.asarray(Wd[hs], np.float32).reshape(-1, DMODEL)),
            "bq": np.ascontiguousarray(np.asarray(bq[hs], np.float32).reshape(-1)),
            "bk": np.ascontiguousarray(np.asarray(bk[hs], np.float32).reshape(-1)),
            "bv": np.ascontiguousarray(np.asarray(bv[hs], np.float32).reshape(-1)),
        })
    return in_maps


def gather_outputs(results, bd):
    """Sum partial outputs per batch and add bd."""
    out = np.zeros((B, L, DMODEL), np.float32)
    per_b = NCORES // B
    for c, res in enumerate(results):
        out[c // per_b] += res["y"]
    out += np.asarray(bd, np.float32)[None, None, :]
    return out


def kernel(x, Wq, bq, Wk, bk, Wv, bv, Wd, bd, _trace=False):
    nc = _get_nc()
    in_maps = shard_inputs(x, Wq, bq, Wk, bk, Wv, bv, Wd, bd)
    res = run_bass_kernel_spmd(nc, in_maps, list(range(NCORES)), trace=_trace)
    out = gather_outputs(res.results, bd)
    if _trace:
        kernel.last_results = res
    return out


# revision 6
# speedup vs baseline: 1.0151x; 1.0151x over previous
"""Trainium2 Bass kernel: multi-head attention (dense transformer block).

Computation (per batch b):
    Q = x @ Wq + bq ; K = x @ Wk + bk ; V = x @ Wv + bv        (per head)
    P = exp((Q @ K^T) / sqrt(Dh))                   (no max-subtraction needed:
                                                     scores are O(1) by construction)
    out = sum_h (P @ V / rowsum(P)) @ Wd[h] + bd

Sharding (data + tensor parallel): 8 cores; core c handles batch b = c // 4
and the 4 heads starting at 4*(c % 4). Each core computes a partial [L, D]
output; the host sums the 4 partials per batch and adds bd.

v2 design notes (per-core):
  - The kernel is a PE/ACT "ridge": matmul stream floor ~143us, exp on the
    scalar(ACT) engine ~128-146us. ScalarE therefore runs NOTHING but the
    128 exp instructions; every drain/cast/copy lives on DVE or GpSimd.
  - All input DMAs are SWDGE (gpsimd) casting fp32->bf16 in flight: no
    staging tiles, no cast instructions. Issue order = priority: biases,
    Wk/Wq(pair0), x tiles (with pair1 / Wv / Wd slotted between), so the
    first K-projection starts ~8us in and the first exp ~11us in.
  - x^T built on-chip via PE transposes (128x128, identity rhs), DVE drain.
  - Per pair: K^T then Q^T chunks (bf16, dual-head 64-row stacking), scores
    S^T = K^T.T @ Q^T land softmax-axis-on-partitions; two heads run on
    independent 64-row PE tiles concurrently. exp on ScalarE (scale=1/8
    fused), 1024 wide, PSUM->SBUF bf16.
  - V for BOTH pairs in one pass ([l',d] layout, N=256 matmuls); drain is a
    single strided DVE add that also applies bv and interleaves with the
    ones-columns used for the denominator trick.
  - attend: O^T = [V_h | ones].T @ P^T; PSUM rows 64..127 = rowsum(P),
    broadcast for free. Normalize: DVE copy + reciprocal_approx_fast
    (~5x faster than the iterative divide) + GpSimd multiply.
  - out-proj is interleaved: as each pair-1 512-chunk normalizes, the
    covered l-tiles' Y = O^T.T @ Wd fire, DVE-drain, and DMA out, so the
    tail is one chunk deep instead of the whole projection.
All matmul operands are bf16 (fp32 accumulation in PSUM).
"""

import os
import sys
from contextlib import ExitStack

import numpy as np

for _p in ("/opt/trn_rl_repo", "/root/.axon_site/_ro/trn_rl_repo"):
    if os.path.isdir(_p) and _p not in sys.path:
        sys.path.append(_p)

import concourse.bass as bass
import concourse.tile as tile
from concourse import bacc, mybir
from concourse.bass import ds, ts
from concourse.bass_utils import run_bass_kernel_spmd
from concourse.masks import make_identity

F32 = mybir.dt.float32
BF16 = mybir.dt.bfloat16

# Problem sizes (hardcoded per contract).
DMODEL, HEADS, DHEAD = 1024, 16, 64
B, L = 2, 2048
NCORES = 8
H_PER_CORE = B * HEADS // NCORES          # 4 heads per core
NPAIR = H_PER_CORE // 2                   # head pairs per core
P = 128                                   # partitions
KT = DMODEL // P                          # 8 k-tiles over dmodel
NLT = L // P                              # 16 l-tiles
LCH = 512                                 # matmul free-dim chunk (one psum bank)
ECH = 1024                                # exp chunk (2 psum banks)
NEC = L // ECH                            # 2 exp chunks
MCH = 512                                 # m-chunk for out-proj
NMC = DMODEL // MCH
PT_BUFS = 40                              # score-tile ring (SBUF budget bound)


def build_nc():
    """Build the SPMD Bass program for one core."""
    nc = bacc.Bacc("TRN2", target_bir_lowering=False, debug=False,
                   num_devices=NCORES)

    x_d = nc.dram_tensor("x", [L, DMODEL], F32, kind="ExternalInput").ap()
    wq_d = nc.dram_tensor("wq", [DMODEL, H_PER_CORE * DHEAD], F32, kind="ExternalInput").ap()
    wk_d = nc.dram_tensor("wk", [DMODEL, H_PER_CORE * DHEAD], F32, kind="ExternalInput").ap()
    wv_d = nc.dram_tensor("wv", [DMODEL, H_PER_CORE * DHEAD], F32, kind="ExternalInput").ap()
    wd_d = nc.dram_tensor("wd", [H_PER_CORE * DHEAD, DMODEL], F32, kind="ExternalInput").ap()
    bq_d = nc.dram_tensor("bq", [H_PER_CORE * DHEAD], F32, kind="ExternalInput").ap()
    bk_d = nc.dram_tensor("bk", [H_PER_CORE * DHEAD], F32, kind="ExternalInput").ap()
    bv_d = nc.dram_tensor("bv", [H_PER_CORE * DHEAD], F32, kind="ExternalInput").ap()
    y_d = nc.dram_tensor("y", [L, DMODEL], F32, kind="ExternalOutput").ap()

    with ExitStack() as ctx:
        tc = ctx.enter_context(tile.TileContext(nc))
        _body(nc, tc, ctx, x_d, wq_d, wk_d, wv_d, wd_d, bq_d, bk_d, bv_d, y_d)
    nc.compile()
    return nc


def _body(nc, tc, ctx, x_d, wq_d, wk_d, wv_d, wd_d, bq_d, bk_d, bv_d, y_d):
    const = ctx.enter_context(tc.tile_pool(name="const", bufs=1))
    sb = ctx.enter_context(tc.tile_pool(name="sb", bufs=1))
    psum = ctx.enter_context(tc.tile_pool(name="psum", bufs=1, space="PSUM"))

    ident = const.tile([P, P], BF16)
    make_identity(nc, ident)

    # ---- SWDGE input DMAs; issue order is the priority order ----
    # biases: [P, {q,k}, pair] per-partition scalars for the q/k drains
    bias_sb = const.tile([P, 2, NPAIR], F32)
    for i, b_d in enumerate((bq_d, bk_d)):
        for p in range(NPAIR):
            nc.gpsimd.dma_start(bias_sb[:, i, p:p + 1],
                                b_d.rearrange("(a p) -> a p", p=P)[p:p + 1, :]
                                .rearrange("a p -> p a"))
    # bv replicated across partitions, (pair, head, d) order = bv flat order
    bv_rep = const.tile([P, NPAIR * P], F32)
    nc.gpsimd.dma_start(bv_rep, bass.AP(tensor=bv_d.tensor, offset=0,
                                        ap=[[0, P], [1, NPAIR * P]]))

    w_sb = const.tile([P, NPAIR, 3, KT, P], BF16)   # [k, pair, {q,k,v}, kt, cols]
    wd_sb = const.tile([P, NPAIR, DMODEL], BF16)

    def w_dma(p, i, w_d):
        nc.gpsimd.dma_start(
            w_sb[:, p, i],
            w_d.rearrange("(kt k) m -> k kt m", k=P)[:, :, ds(p * P, P)])

    w_dma(0, 1, wk_d)   # pair-0 K first: first projection to run
    w_dma(0, 0, wq_d)

    # shared [128,1024] psum slots for scores & transposes
    def sc_tile(shape=None, dt=F32):
        return psum.tile(shape or [P, ECH], dt, tag="sctr", bufs=2, name="sctr")

    # ---- phase 0: x cast-DMA (fp32->bf16 in flight) + PE transposes ----
    xt = sb.tile([P, KT, L], BF16)
    for lt in range(NLT):
        xb = sb.tile([P, DMODEL], BF16, tag="xb", bufs=3)
        nc.gpsimd.dma_start(xb, x_d[ds(lt * P, P), :])
        if lt == 3:
            w_dma(1, 1, wk_d)
            w_dma(1, 0, wq_d)
        elif lt == 7:
            w_dma(0, 2, wv_d)
            w_dma(1, 2, wv_d)
        elif lt == 11:
            nc.gpsimd.dma_start(wd_sb, wd_d.rearrange("(pp k) m -> k pp m", k=P))
        tp = sc_tile([P, KT, P], BF16)
        for kt in range(KT):
            nc.tensor.transpose(tp[:, kt], xb[:, ds(kt * P, P)], ident)
        nc.vector.tensor_copy(xt[:, :, ds(lt * P, P)], tp)

    o_norm = sb.tile([P, NPAIR, L], BF16)
    # V for both pairs in [l', (pair,head)|{v,ones}] interleaved layout
    vt = sb.tile([P, NLT, 2 * NPAIR, 2, DHEAD], BF16)

    # ---- per pair: K -> Q(chunk0) -> scores/exp/attend blocks ----
    def qkv_proj(dst, p, i, lcs):
        for lc in lcs:
            ps = psum.tile([P, LCH], F32, tag="qkvp", bufs=2, name="qkvps")
            for kt in range(KT):
                nc.tensor.matmul(
                    ps, lhsT=w_sb[:, p, i, kt],
                    rhs=xt[:, kt, ds(lc * LCH, LCH)],
                    start=(kt == 0), stop=(kt == KT - 1))
            nc.vector.tensor_scalar_add(
                dst[:, ds(lc * LCH, LCH)], ps, bias_sb[:, i, p:p + 1])

    for p in range(NPAIR):
        qT = sb.tile([P, L], BF16, tag="qkv0", bufs=NPAIR)
        kT_sb = sb.tile([P, L], BF16, tag="qkv1", bufs=NPAIR)
        qkv_proj(kT_sb, p, 1, range(L // LCH))
        qkv_proj(qT, p, 0, range(ECH // LCH))

        for ec in range(NEC):
            if ec > 0:
                qkv_proj(qT, p, 0, range(ec * ECH // LCH, (ec + 1) * ECH // LCH))
            pt_tiles = [[None] * NLT, [None] * NLT]
            for lt in range(NLT):
                for h in range(2):
                    sp = sc_tile()
                    for sub in range(ECH // LCH):
                        nc.tensor.matmul(
                            sp[:, ds(sub * LCH, LCH)],
                            lhsT=kT_sb[ds(64 * h, 64), ds(lt * P, P)],
                            rhs=qT[ds(64 * h, 64),
                                   ds(ec * ECH + sub * LCH, LCH)],
                            start=True, stop=True)
                    pt = sb.tile([P, ECH], BF16, tag="pt", bufs=PT_BUFS)
                    nc.scalar.activation(
                        pt, sp, func=mybir.ActivationFunctionType.Exp,
                        scale=1.0 / np.sqrt(DHEAD))
                    pt_tiles[h][lt] = pt

            if p == 0 and ec == 0:
                # V (both pairs) in [l', d] layout, emitted after the first
                # scores so the exp pipeline starts as early as possible
                nc.vector.memset(vt[:, :, :, 1, :], 1.0)
                for lt in range(NLT):
                    vp = psum.tile([P, 2 * NPAIR * DHEAD], F32,
                                   tag="qkvp", bufs=2, name="vp")
                    for kt in range(KT):
                        nc.tensor.matmul(
                            vp, lhsT=xt[:, kt, ds(lt * P, P)],
                            rhs=w_sb[:, :, 2, kt, :],
                            start=(kt == 0), stop=(kt == KT - 1))
                    nc.vector.tensor_add(
                        vt[:, lt, :, 0, :],
                        vp.rearrange("a (g d) -> a g d", d=DHEAD),
                        bv_rep.rearrange("a (g d) -> a g d", d=DHEAD))

            for sub in range(ECH // LCH):
                lc = ec * ECH + sub * LCH
                for h in range(2):
                    op = psum.tile([P, LCH], F32, tag="op", bufs=2)
                    for lt in range(NLT):
                        nc.tensor.matmul(
                            op, lhsT=vt[:, lt, 2 * p + h],
                            rhs=pt_tiles[h][lt][:, ds(sub * LCH, LCH)],
                            start=(lt == 0), stop=(lt == NLT - 1))
                    # rows 64..127 are the denominator, already broadcast.
                    # reciprocal_approx_fast mishandles base_partition != 0,
                    # so the denominators drain to their own partition-0 tile.
                    os_sb = sb.tile([DHEAD, LCH], F32, tag="os", bufs=3)
                    nc.vector.tensor_copy(os_sb, op[0:DHEAD, :])
                    dn = sb.tile([DHEAD, LCH], F32, tag="dn", bufs=3)
                    nc.vector.tensor_copy(dn, op[DHEAD:P, :])
                    rs = sb.tile([DHEAD, LCH], F32, tag="rs", bufs=2)
                    nc.vector.reciprocal_approx_fast(rs, dn)
                    nc.gpsimd.tensor_mul(
                        o_norm[ds(64 * h, 64), p, ds(lc, LCH)],
                        os_sb, rs)

                if p == NPAIR - 1:
                    # out-proj for the l-tiles this chunk completed
                    for lt in range(lc // P, lc // P + LCH // P):
                        for mc in range(NMC):
                            yp = psum.tile([P, MCH], F32, tag="qkvp",
                                           bufs=2, name="yp")
                            for pp in range(NPAIR):
                                nc.tensor.matmul(
                                    yp, lhsT=o_norm[:, pp, ds(lt * P, P)],
                                    rhs=wd_sb[:, pp, ds(mc * MCH, MCH)],
                                    start=(pp == 0), stop=(pp == NPAIR - 1))
                            ys = sb.tile([P, MCH], F32, tag="ys", bufs=3)
                            nc.vector.tensor_copy(ys, yp)
                            nc.sync.dma_start(
                                y_d[ds(lt * P, P), ds(mc * MCH, MCH)], ys)


_NC_CACHE = {}


def _get_nc():
    if "nc" not in _NC_CACHE:
        _NC_CACHE["nc"] = build_nc()
    return _NC_CACHE["nc"]


def shard_inputs(x, Wq, bq, Wk, bk, Wv, bv, Wd, bd):
    """Build the 8 per-core input maps."""
    in_maps = []
    for c in range(NCORES):
        b = c // (NCORES // B)
        h0 = (c % (NCORES // B)) * H_PER_CORE
        hs = slice(h0, h0 + H_PER_CORE)
        in_maps.append({
            "x": np.ascontiguousarray(np.asarray(x[b], np.float32)),
            "wq": np.ascontiguousarray(np.asarray(Wq[:, hs, :], np.float32).reshape(DMODEL, -1)),
            "wk": np.ascontiguousarray(np.asarray(Wk[:, hs, :], np.float32).reshape(DMODEL, -1)),
            "wv": np.ascontiguousarray(np.asarray(Wv[:, hs, :], np.float32).reshape(DMODEL, -1)),
            "wd": np.ascontiguousarray(np.asarray(Wd[hs], np.float32).reshape(-1, DMODEL)),
            "bq": np.ascontiguousarray(np.asarray(bq[hs], np.float32).reshape(-1)),
            "bk": np.ascontiguousarray(np.asarray(bk[hs], np.float32).reshape(-1)),
            "bv": np.ascontiguousarray(np.asarray(bv[hs], np.float32).reshape(-1)),
        })
    return in_maps


def gather_outputs(results, bd):
    """Sum partial outputs per batch and add bd."""
    out = np.zeros((B, L, DMODEL), np.float32)
    per_b = NCORES // B
    for c, res in enumerate(results):
        out[c // per_b] += res["y"]
    out += np.asarray(bd, np.float32)[None, None, :]
    return out


def kernel(x, Wq, bq, Wk, bk, Wv, bv, Wd, bd, _trace=False):
    nc = _get_nc()
    in_maps = shard_inputs(x, Wq, bq, Wk, bk, Wv, bv, Wd, bd)
    res = run_bass_kernel_spmd(nc, in_maps, list(range(NCORES)), trace=_trace)
    out = gather_outputs(res.results, bd)
    if _trace:
        kernel.last_results = res
    return out


# revision 11
# speedup vs baseline: 1.1057x; 1.0892x over previous
"""Trainium2 Bass kernel: multi-head attention (dense transformer block).

Computation (per batch b):
    Q = x @ Wq + bq ; K = x @ Wk + bk ; V = x @ Wv + bv        (per head)
    P = exp((Q @ K^T) / sqrt(Dh))                   (no max-subtraction needed:
                                                     scores are O(1) by construction)
    out = sum_h (P @ V / rowsum(P)) @ Wd[h] + bd

Sharding (data + tensor parallel): 8 cores; core c handles batch b = c // 4
and the 4 heads starting at 4*(c % 4). Each core computes a partial [L, D]
output; the host sums the 4 partials per batch and adds bd.

v3 design notes (per-core):
  - The kernel is a PE/ACT "ridge": matmul stream floor ~143us, exp on the
    scalar(ACT) engine ~146us. ScalarE runs NOTHING but the 128 exp
    instructions; every drain/cast/copy lives on DVE or GpSimd.
  - x is DMA'd fp32 (HWDGE) and PE-transposed in fp32 (the psum slots are
    4KB either way); the mandatory DVE drain to SBUF casts to bf16 for
    free, so no standalone cast instructions exist.
  - The four (pair, ec) chunks are software-pipelined: after each attend
    pass of chunk k, a slice of chunk k+1's scores+exp is emitted, so the
    ACT engine never runs dry at chunk boundaries; the next pair's K/Q
    projections ride the same slots.
  - Per pair: K^T/Q^T bf16 with dual-head 64-row stacking; scores
    S^T = K^T.T @ Q^T land softmax-axis-on-partitions; the two heads run
    on independent 64-row PE tiles concurrently. exp on ScalarE
    (scale=1/8 fused), 1024 wide, PSUM->SBUF bf16.
  - V for BOTH pairs in one pass ([l',d] layout, N=256 matmuls); its DVE
    drain applies bv and interleaves the ones-columns of the denominator
    trick.
  - attend: O^T = [V_h | ones].T @ P^T; PSUM rows 64..127 = rowsum(P),
    broadcast for free. Normalize: DVE drains + reciprocal_approx_fast
    (fed from a partition-0 tile; the custom op mishandles base!=0) +
    GpSimd multiply.
  - out-proj is interleaved: as each pair-1 512-chunk normalizes, the
    covered l-tiles' Y = O^T.T @ Wd fire (after the next chunk's score
    slice, to keep exp fed), DVE-drain, and DMA out.
All matmul operands are bf16 (fp32 accumulation in PSUM).
"""

import os
import sys
from contextlib import ExitStack

import numpy as np

for _p in ("/opt/trn_rl_repo", "/root/.axon_site/_ro/trn_rl_repo"):
    if os.path.isdir(_p) and _p not in sys.path:
        sys.path.append(_p)

import concourse.bass as bass
import concourse.tile as tile
from concourse import bacc, mybir
from concourse.bass import ds, ts
from concourse.bass_utils import run_bass_kernel_spmd
from concourse.masks import make_identity

F32 = mybir.dt.float32
BF16 = mybir.dt.bfloat16

# Problem sizes (hardcoded per contract).
DMODEL, HEADS, DHEAD = 1024, 16, 64
B, L = 2, 2048
NCORES = 8
H_PER_CORE = B * HEADS // NCORES          # 4 heads per core
NPAIR = H_PER_CORE // 2                   # head pairs per core
P = 128                                   # partitions
KT = DMODEL // P                          # 8 k-tiles over dmodel
NLT = L // P                              # 16 l-tiles
LCH = 512                                 # matmul free-dim chunk (one psum bank)
ECH = 1024                                # exp chunk (2 psum banks)
NEC = L // ECH                            # 2 exp chunks
MCH = 512                                 # m-chunk for out-proj
NMC = DMODEL // MCH
PT_BUFS = 40                              # score-tile ring (SBUF budget bound)


def build_nc():
    """Build the SPMD Bass program for one core."""
    nc = bacc.Bacc("TRN2", target_bir_lowering=False, debug=False,
                   num_devices=NCORES)

    x_d = nc.dram_tensor("x", [L, DMODEL], F32, kind="ExternalInput").ap()
    wq_d = nc.dram_tensor("wq", [DMODEL, H_PER_CORE * DHEAD], F32, kind="ExternalInput").ap()
    wk_d = nc.dram_tensor("wk", [DMODEL, H_PER_CORE * DHEAD], F32, kind="ExternalInput").ap()
    wv_d = nc.dram_tensor("wv", [DMODEL, H_PER_CORE * DHEAD], F32, kind="ExternalInput").ap()
    wd_d = nc.dram_tensor("wd", [H_PER_CORE * DHEAD, DMODEL], F32, kind="ExternalInput").ap()
    bq_d = nc.dram_tensor("bq", [H_PER_CORE * DHEAD], F32, kind="ExternalInput").ap()
    bk_d = nc.dram_tensor("bk", [H_PER_CORE * DHEAD], F32, kind="ExternalInput").ap()
    bv_d = nc.dram_tensor("bv", [H_PER_CORE * DHEAD], F32, kind="ExternalInput").ap()
    y_d = nc.dram_tensor("y", [L, DMODEL], F32, kind="ExternalOutput").ap()

    with ExitStack() as ctx:
        tc = ctx.enter_context(tile.TileContext(nc))
        _body(nc, tc, ctx, x_d, wq_d, wk_d, wv_d, wd_d, bq_d, bk_d, bv_d, y_d)
    nc.compile()
    return nc


def _body(nc, tc, ctx, x_d, wq_d, wk_d, wv_d, wd_d, bq_d, bk_d, bv_d, y_d):
    const = ctx.enter_context(tc.tile_pool(name="const", bufs=1))
    sb = ctx.enter_context(tc.tile_pool(name="sb", bufs=1))
    psum = ctx.enter_context(tc.tile_pool(name="psum", bufs=1, space="PSUM"))

    identf = const.tile([P, P], F32)
    make_identity(nc, identf)

    # biases: [P, {q,k}, pair] per-partition scalars for the q/k drains
    bias_sb = const.tile([P, 2, NPAIR], F32)
    for i, b_d in enumerate((bq_d, bk_d)):
        for p in range(NPAIR):
            nc.gpsimd.dma_start(bias_sb[:, i, p:p + 1],
                                b_d.rearrange("(a p) -> a p", p=P)[p:p + 1, :]
                                .rearrange("a p -> p a"))
    # bv replicated across partitions, (pair, head, d) order = bv flat order
    bv_rep = const.tile([P, NPAIR * P], F32)
    nc.gpsimd.dma_start(bv_rep, bass.AP(tensor=bv_d.tensor, offset=0,
                                        ap=[[0, P], [1, NPAIR * P]]))

    # weights: fp32 HWDGE DMA into staging, DVE cast into bf16 tiles
    w_sb = const.tile([P, NPAIR, 3, KT, P], BF16)   # [k, pair, {q,k,v}, kt, cols]
    wd_sb = const.tile([P, NPAIR, DMODEL], BF16)

    def w_load(p, i, w_d):
        ws = sb.tile([P, KT, P], F32, tag="wstage", bufs=2)
        nc.sync.dma_start(
            ws, w_d.rearrange("(kt k) m -> k kt m", k=P)[:, :, ds(p * P, P)])
        nc.vector.tensor_copy(w_sb[:, p, i], ws)

    w_load(0, 1, wk_d)   # pair-0 K first: first projection to run
    w_load(0, 0, wq_d)

    # shared [128,1024] psum slots for scores & transposes (both 4KB fp32)
    def sc_tile(shape=None):
        return psum.tile(shape or [P, ECH], F32, tag="sctr", bufs=2, name="sctr")

    # ---- phase 0: x fp32 in, PE fp32 transposes, casting DVE drain ----
    xt = sb.tile([P, KT, L], BF16)
    for lt in range(NLT):
        xs = sb.tile([P, DMODEL], F32, tag="xs", bufs=3)
        nc.sync.dma_start(xs, x_d[ds(lt * P, P), :])
        if lt == 3:
            w_load(1, 1, wk_d)
            w_load(1, 0, wq_d)
        elif lt == 7:
            w_load(0, 2, wv_d)
            w_load(1, 2, wv_d)
        elif lt in (11, 12):
            pp = lt - 11
            wds = sb.tile([P, DMODEL], F32, tag="wstage", bufs=2, name="wds")
            nc.sync.dma_start(
                wds, wd_d.rearrange("(pp k) m -> k pp m", k=P)[:, pp])
            nc.vector.tensor_copy(wd_sb[:, pp], wds)
        tp = sc_tile([P, KT, P])
        for kt in range(KT):
            nc.tensor.transpose(tp[:, kt], xs[:, ds(kt * P, P)], identf)
        nc.vector.tensor_copy(xt[:, :, ds(lt * P, P)], tp)

    o_norm = sb.tile([P, NPAIR, L], BF16)
    # V for both pairs in [l', (pair,head)|{v,ones}] interleaved layout
    vt = sb.tile([P, NLT, 2 * NPAIR, 2, DHEAD], BF16)

    def qkv_proj(dst, p, i, lcs):
        for lc in lcs:
            ps = psum.tile([P, LCH], F32, tag="qkvp", bufs=2, name="qkvps")
            for kt in range(KT):
                nc.tensor.matmul(
                    ps, lhsT=w_sb[:, p, i, kt],
                    rhs=xt[:, kt, ds(lc * LCH, LCH)],
                    start=(kt == 0), stop=(kt == KT - 1))
            nc.vector.tensor_scalar_add(
                dst[:, ds(lc * LCH, LCH)], ps, bias_sb[:, i, p:p + 1])

    def emit_v():
        nc.vector.memset(vt[:, :, :, 1, :], 1.0)
        for lt in range(NLT):
            vp = psum.tile([P, 2 * NPAIR * DHEAD], F32,
                           tag="qkvp", bufs=2, name="vp")
            for kt in range(KT):
                nc.tensor.matmul(
                    vp, lhsT=xt[:, kt, ds(lt * P, P)],
                    rhs=w_sb[:, :, 2, kt, :],
                    start=(kt == 0), stop=(kt == KT - 1))
            nc.vector.tensor_add(
                vt[:, lt, :, 0, :],
                vp.rearrange("a (g d) -> a g d", d=DHEAD),
                bv_rep.rearrange("a (g d) -> a g d", d=DHEAD))

    # per-(pair, ec) chunk state: qT/kT tiles and the pt ring slices
    qT = {}
    kT = {}
    pt_tiles = {}

    def emit_scores(p, ec, lts):
        for lt in lts:
            for h in range(2):
                sp = sc_tile()
                for sub in range(ECH // LCH):
                    nc.tensor.matmul(
                        sp[:, ds(sub * LCH, LCH)],
                        lhsT=kT[p][ds(64 * h, 64), ds(lt * P, P)],
                        rhs=qT[p][ds(64 * h, 64),
                                  ds(ec * ECH + sub * LCH, LCH)],
                        start=True, stop=True)
                pt = sb.tile([P, ECH], BF16, tag="pt", bufs=PT_BUFS)
                nc.scalar.activation(
                    pt, sp, func=mybir.ActivationFunctionType.Exp,
                    scale=1.0 / np.sqrt(DHEAD))
                pt_tiles[(p, ec)][h][lt] = pt

    def emit_attend_pass(p, ec, sub, h):
        lc = ec * ECH + sub * LCH
        op = psum.tile([P, LCH], F32, tag="op", bufs=2)
        for lt in range(NLT):
            nc.tensor.matmul(
                op, lhsT=vt[:, lt, 2 * p + h],
                rhs=pt_tiles[(p, ec)][h][lt][:, ds(sub * LCH, LCH)],
                start=(lt == 0), stop=(lt == NLT - 1))
        # rows 64..127 are the denominator, already broadcast.
        # reciprocal_approx_fast mishandles base_partition != 0, so the
        # denominators drain to their own partition-0 tile.
        os_sb = sb.tile([DHEAD, LCH], F32, tag="os", bufs=2)
        nc.vector.tensor_copy(os_sb, op[0:DHEAD, :])
        dn = sb.tile([DHEAD, LCH], F32, tag="dn", bufs=2)
        nc.vector.tensor_copy(dn, op[DHEAD:P, :])
        rs = sb.tile([DHEAD, LCH], F32, tag="rs", bufs=2)
        nc.vector.reciprocal_approx_fast(rs, dn)
        nc.gpsimd.tensor_mul(
            o_norm[ds(64 * h, 64), p, ds(lc, LCH)], os_sb, rs)

    def emit_outproj(lc):
        for lt in range(lc // P, lc // P + LCH // P):
            for mc in range(NMC):
                yp = psum.tile([P, MCH], F32, tag="qkvp", bufs=2, name="yp")
                for pp in range(NPAIR):
                    nc.tensor.matmul(
                        yp, lhsT=o_norm[:, pp, ds(lt * P, P)],
                        rhs=wd_sb[:, pp, ds(mc * MCH, MCH)],
                        start=(pp == 0), stop=(pp == NPAIR - 1))
                ys = sb.tile([P, MCH], F32, tag="ys", bufs=3)
                nc.vector.tensor_copy(ys, yp)
                nc.sync.dma_start(
                    y_d[ds(lt * P, P), ds(mc * MCH, MCH)], ys)

    PASSES = [(sub, h) for sub in range(ECH // LCH) for h in range(2)]

    # ---- pipeline schedule over the four (pair, ec) chunks ----
    for p in range(NPAIR):
        qT[p] = sb.tile([P, L], BF16, tag="qkv0", bufs=NPAIR, name=f"qT{p}")
        kT[p] = sb.tile([P, L], BF16, tag="qkv1", bufs=NPAIR, name=f"kT{p}")
    for key in [(p, ec) for p in range(NPAIR) for ec in range(NEC)]:
        pt_tiles[key] = [[None] * NLT, [None] * NLT]

    qkv_proj(kT[0], 0, 1, range(L // LCH))
    qkv_proj(qT[0], 0, 0, range(ECH // LCH))
    emit_scores(0, 0, range(NLT))
    emit_v()
    qkv_proj(qT[0], 0, 0, range(ECH // LCH, 2 * ECH // LCH))

    # chunk (0,0) attend ∥ chunk (0,1) scores
    for j, (sub, h) in enumerate(PASSES):
        emit_attend_pass(0, 0, sub, h)
        if j >= 2:
            emit_scores(0, 1, range(4 * (j - 2), 4 * (j - 1)))
    emit_scores(0, 1, range(8, NLT))

    # chunk (0,1) attend ∥ pair-1 K/Q projections + first chunk (1,0) scores
    for j, (sub, h) in enumerate(PASSES):
        emit_attend_pass(0, 1, sub, h)
        if j == 0:
            qkv_proj(kT[1], 1, 1, range(2))
        elif j == 1:
            qkv_proj(kT[1], 1, 1, range(2, 4))
        elif j == 2:
            qkv_proj(qT[1], 1, 0, range(ECH // LCH))
        else:
            emit_scores(1, 0, range(4))
    emit_scores(1, 0, range(4, NLT))
    qkv_proj(qT[1], 1, 0, range(ECH // LCH, 2 * ECH // LCH))

    # chunk (1,0) attend ∥ chunk (1,1) scores; out-proj rides behind
    for j, (sub, h) in enumerate(PASSES):
        emit_attend_pass(1, 0, sub, h)
        if j >= 2:
            emit_scores(1, 1, range(4 * (j - 2), 4 * (j - 1)))
        if h == 1:
            emit_outproj(sub * LCH)
    emit_scores(1, 1, range(8, NLT))

    for j, (sub, h) in enumerate(PASSES):
        emit_attend_pass(1, 1, sub, h)
        if h == 1:
            emit_outproj(ECH + sub * LCH)


_NC_CACHE = {}


def _get_nc():
    if "nc" not in _NC_CACHE:
        _NC_CACHE["nc"] = build_nc()
    return _NC_CACHE["nc"]


def shard_inputs(x, Wq, bq, Wk, bk, Wv, bv, Wd, bd):
    """Build the 8 per-core input maps."""
    in_maps = []
    for c in range(NCORES):
        b = c // (NCORES // B)
        h0 = (c % (NCORES // B)) * H_PER_CORE
        hs = slice(h0, h0 + H_PER_CORE)
        in_maps.append({
            "x": np.ascontiguousarray(np.asarray(x[b], np.float32)),
            "wq": np.ascontiguousarray(np.asarray(Wq[:, hs, :], np.float32).reshape(DMODEL, -1)),
            "wk": np.ascontiguousarray(np.asarray(Wk[:, hs, :], np.float32).reshape(DMODEL, -1)),
            "wv": np.ascontiguousarray(np.asarray(Wv[:, hs, :], np.float32).reshape(DMODEL, -1)),
            "wd": np.ascontiguousarray(np.asarray(Wd[hs], np.float32).reshape(-1, DMODEL)),
            "bq": np.ascontiguousarray(np.asarray(bq[hs], np.float32).reshape(-1)),
            "bk": np.ascontiguousarray(np.asarray(bk[hs], np.float32).reshape(-1)),
            "bv": np.ascontiguousarray(np.asarray(bv[hs], np.float32).reshape(-1)),
        })
    return in_maps


def gather_outputs(results, bd):
    """Sum partial outputs per batch and add bd."""
    out = np.zeros((B, L, DMODEL), np.float32)
    per_b = NCORES // B
    for c, res in enumerate(results):
        out[c // per_b] += res["y"]
    out += np.asarray(bd, np.float32)[None, None, :]
    return out


def kernel(x, Wq, bq, Wk, bk, Wv, bv, Wd, bd, _trace=False):
    nc = _get_nc()
    in_maps = shard_inputs(x, Wq, bq, Wk, bk, Wv, bv, Wd, bd)
    res = run_bass_kernel_spmd(nc, in_maps, list(range(NCORES)), trace=_trace)
    out = gather_outputs(res.results, bd)
    if _trace:
        kernel.last_results = res
    return out


# revision 12
# speedup vs baseline: 1.1928x; 1.0788x over previous
"""Trainium2 Bass kernel: multi-head attention (dense transformer block).

Computation (per batch b):
    Q = x @ Wq + bq ; K = x @ Wk + bk ; V = x @ Wv + bv        (per head)
    P = exp((Q @ K^T) / sqrt(Dh))                   (no max-subtraction needed:
                                                     scores are O(1) by construction)
    out = sum_h (P @ V / rowsum(P)) @ Wd[h] + bd

Sharding (data + tensor parallel): 8 cores; core c handles batch b = c // 4
and the 4 heads starting at 4*(c % 4). Each core computes a partial [L, D]
output; the host sums the 4 partials per batch and adds bd.

v4 design notes (per-core):
  - The kernel is a PE/ACT "ridge": matmul stream floor ~143us, exp on the
    scalar(ACT) engine ~146us irreducible. ScalarE runs NOTHING but the 128
    exp instructions; drains/casts/copies live on DVE or GpSimd.
  - Four (pair, ec) chunks are software-pipelined per l-tile: window k runs
    attend(chunk k) and scores+exp(chunk k+1) interleaved at l-tile
    granularity, so the ACT engine never waits at chunk boundaries. The
    attend uses 4 PSUM accumulators (one per (sub, h) stream), freeing a
    full 4-bank double-buffered score slot; a "normalize burst" at each
    window boundary drains all four, while the PE runs the next chunk's
    K/Q projections (or the out-projection) in the gap.
  - x DMAs stream on the sync HWDGE queue; weights ride the scalar HWDGE
    queue in parallel, so the first K-projection starts ~5us in.
  - Per pair: K^T/Q^T bf16 with dual-head 64-row stacking; scores
    S^T = K^T.T @ Q^T land softmax-axis-on-partitions; the two heads run
    on independent 64-row PE tiles concurrently. exp on ScalarE
    (scale=1/8 fused), 1024 wide, PSUM->SBUF bf16.
  - V for BOTH pairs in one pass ([l',d] layout, N=256 matmuls); its DVE
    drain applies bv and interleaves the ones-columns of the denominator
    trick.
  - attend: O^T = [V_h | ones].T @ P^T; PSUM rows 64..127 = rowsum(P),
    broadcast for free. Normalize: DVE drains + reciprocal_approx_fast
    (fed from a partition-0 tile; the custom op mishandles base!=0) +
    GpSimd multiply.
All matmul operands are bf16 (fp32 accumulation in PSUM).
"""

import os
import sys
from contextlib import ExitStack

import numpy as np

for _p in ("/opt/trn_rl_repo", "/root/.axon_site/_ro/trn_rl_repo"):
    if os.path.isdir(_p) and _p not in sys.path:
        sys.path.append(_p)

import concourse.bass as bass
import concourse.tile as tile
from concourse import bacc, mybir
from concourse.bass import ds, ts
from concourse.bass_utils import run_bass_kernel_spmd
from concourse.masks import make_identity

F32 = mybir.dt.float32
BF16 = mybir.dt.bfloat16

# Problem sizes (hardcoded per contract).
DMODEL, HEADS, DHEAD = 1024, 16, 64
B, L = 2, 2048
NCORES = 8
H_PER_CORE = B * HEADS // NCORES          # 4 heads per core
NPAIR = H_PER_CORE // 2                   # head pairs per core
P = 128                                   # partitions
KT = DMODEL // P                          # 8 k-tiles over dmodel
NLT = L // P                              # 16 l-tiles
LCH = 512                                 # matmul free-dim chunk (one psum bank)
ECH = 1024                                # exp chunk (2 psum banks)
NEC = L // ECH                            # 2 exp chunks
MCH = 512                                 # m-chunk for out-proj
NMC = DMODEL // MCH
PT_BUFS = 39                              # score-tile ring (SBUF budget bound)
CHUNKS = [(p, ec) for p in range(NPAIR) for ec in range(NEC)]


def build_nc():
    """Build the SPMD Bass program for one core."""
    nc = bacc.Bacc("TRN2", target_bir_lowering=False, debug=False,
                   num_devices=NCORES)

    x_d = nc.dram_tensor("x", [L, DMODEL], F32, kind="ExternalInput").ap()
    wq_d = nc.dram_tensor("wq", [DMODEL, H_PER_CORE * DHEAD], F32, kind="ExternalInput").ap()
    wk_d = nc.dram_tensor("wk", [DMODEL, H_PER_CORE * DHEAD], F32, kind="ExternalInput").ap()
    wv_d = nc.dram_tensor("wv", [DMODEL, H_PER_CORE * DHEAD], F32, kind="ExternalInput").ap()
    wd_d = nc.dram_tensor("wd", [H_PER_CORE * DHEAD, DMODEL], F32, kind="ExternalInput").ap()
    bq_d = nc.dram_tensor("bq", [H_PER_CORE * DHEAD], F32, kind="ExternalInput").ap()
    bk_d = nc.dram_tensor("bk", [H_PER_CORE * DHEAD], F32, kind="ExternalInput").ap()
    bv_d = nc.dram_tensor("bv", [H_PER_CORE * DHEAD], F32, kind="ExternalInput").ap()
    y_d = nc.dram_tensor("y", [L, DMODEL], F32, kind="ExternalOutput").ap()

    with ExitStack() as ctx:
        tc = ctx.enter_context(tile.TileContext(nc))
        _body(nc, tc, ctx, x_d, wq_d, wk_d, wv_d, wd_d, bq_d, bk_d, bv_d, y_d)
    nc.compile()
    return nc


def _body(nc, tc, ctx, x_d, wq_d, wk_d, wv_d, wd_d, bq_d, bk_d, bv_d, y_d):
    const = ctx.enter_context(tc.tile_pool(name="const", bufs=1))
    sb = ctx.enter_context(tc.tile_pool(name="sb", bufs=1))
    psum = ctx.enter_context(tc.tile_pool(name="psum", bufs=1, space="PSUM"))

    ident = const.tile([P, P], BF16)
    make_identity(nc, ident)

    # biases (tiny, SWDGE): [P, {q,k}, pair] per-partition scalars
    bias_sb = const.tile([P, 2, NPAIR], F32)
    for i, b_d in enumerate((bq_d, bk_d)):
        for p in range(NPAIR):
            nc.gpsimd.dma_start(bias_sb[:, i, p:p + 1],
                                b_d.rearrange("(a p) -> a p", p=P)[p:p + 1, :]
                                .rearrange("a p -> p a"))
    bv_rep = const.tile([P, NPAIR * P], F32)
    nc.gpsimd.dma_start(bv_rep, bass.AP(tensor=bv_d.tensor, offset=0,
                                        ap=[[0, P], [1, NPAIR * P]]))

    # weights: fp32 on the SCALAR HWDGE queue (parallel to x on sync), DVE cast
    w_sb = const.tile([P, NPAIR, 3, KT, P], BF16)   # [k, pair, {q,k,v}, kt, cols]
    wd_sb = const.tile([P, NPAIR, DMODEL], BF16)

    def w_load(p, i, w_d):
        ws = sb.tile([P, KT, P], F32, tag="wstage", bufs=2)
        nc.scalar.dma_start(
            ws, w_d.rearrange("(kt k) m -> k kt m", k=P)[:, :, ds(p * P, P)])
        nc.vector.tensor_copy(w_sb[:, p, i], ws)

    w_load(0, 1, wk_d)   # pair-0 K first: first projection to run
    w_load(0, 0, wq_d)
    w_load(1, 1, wk_d)
    w_load(1, 0, wq_d)
    w_load(0, 2, wv_d)
    w_load(1, 2, wv_d)
    for pp in range(NPAIR):
        wds = sb.tile([P, DMODEL], F32, tag="wstage", bufs=2, name="wds")
        nc.scalar.dma_start(
            wds, wd_d.rearrange("(pp k) m -> k pp m", k=P)[:, pp])
        nc.vector.tensor_copy(wd_sb[:, pp], wds)

    # PSUM: "sp" 2x[P,1024]f32 (4 banks) for scores; "op" 4x[P,512]f32
    # (4 banks) for the attend accumulators, timeshared at boundaries by
    # projections / transposes / out-proj.
    def op_tile(shape=None, dt=F32, name="opx"):
        return psum.tile(shape or [P, LCH], dt, tag="op", bufs=4, name=name)

    o_norm = sb.tile([P, NPAIR, L], BF16)
    vt = sb.tile([P, NLT, 2 * NPAIR, 2, DHEAD], BF16)
    xt = sb.tile([P, KT, L], BF16)

    def qkv_proj(dst, p, i, lcs):
        for lc in lcs:
            ps = op_tile(name="qkvps")
            for kt in range(KT):
                nc.tensor.matmul(
                    ps, lhsT=w_sb[:, p, i, kt],
                    rhs=xt[:, kt, ds(lc * LCH, LCH)],
                    start=(kt == 0), stop=(kt == KT - 1))
            nc.vector.tensor_scalar_add(
                dst[:, ds(lc * LCH, LCH)], ps, bias_sb[:, i, p:p + 1])

    # ---- phase 0: x fp32 in (sync queue), DVE cast, PE transposes ----
    qT = {}
    kT = {}
    for p in range(NPAIR):
        qT[p] = sb.tile([P, L], BF16, tag="qkv0", bufs=NPAIR, name=f"qT{p}")
        kT[p] = sb.tile([P, L], BF16, tag="qkv1", bufs=NPAIR, name=f"kT{p}")

    for g in range(4):
        for lt in range(4 * g, 4 * g + 4):
            xs = sb.tile([P, DMODEL], F32, tag="xs", bufs=3)
            nc.sync.dma_start(xs, x_d[ds(lt * P, P), :])
            xb = sb.tile([P, DMODEL], BF16, tag="xb", bufs=2)
            nc.vector.tensor_copy(xb, xs)
            tp = op_tile([P, KT, P], BF16, name="tp")
            for kt in range(KT):
                nc.tensor.transpose(tp[:, kt], xb[:, ds(kt * P, P)], ident)
            nc.vector.tensor_copy(xt[:, :, ds(lt * P, P)], tp)
        qkv_proj(kT[0], 0, 1, [g])
        if g < 2:
            qkv_proj(qT[0], 0, 0, [g])

    pt_tiles = {key: [[None] * NLT, [None] * NLT] for key in CHUNKS}

    def emit_scores(p, ec, lts):
        for lt in lts:
            for h in range(2):
                sp = psum.tile([P, ECH], F32, tag="sp", bufs=2, name="sp")
                for sub in range(ECH // LCH):
                    nc.tensor.matmul(
                        sp[:, ds(sub * LCH, LCH)],
                        lhsT=kT[p][ds(64 * h, 64), ds(lt * P, P)],
                        rhs=qT[p][ds(64 * h, 64),
                                  ds(ec * ECH + sub * LCH, LCH)],
                        start=True, stop=True)
                pt = sb.tile([P, ECH], BF16, tag="pt", bufs=PT_BUFS)
                nc.scalar.activation(
                    pt, sp, func=mybir.ActivationFunctionType.Exp,
                    scale=1.0 / np.sqrt(DHEAD))
                pt_tiles[(p, ec)][h][lt] = pt

    def emit_v():
        nc.vector.memset(vt[:, :, :, 1, :], 1.0)
        for lt in range(NLT):
            vp = op_tile([P, 2 * NPAIR * DHEAD], name="vp")
            for kt in range(KT):
                nc.tensor.matmul(
                    vp, lhsT=xt[:, kt, ds(lt * P, P)],
                    rhs=w_sb[:, :, 2, kt, :],
                    start=(kt == 0), stop=(kt == KT - 1))
            nc.vector.tensor_add(
                vt[:, lt, :, 0, :],
                vp.rearrange("a (g d) -> a g d", d=DHEAD),
                bv_rep.rearrange("a (g d) -> a g d", d=DHEAD))

    def norm_one(p, ec, sub, h, op):
        lc = ec * ECH + sub * LCH
        os_sb = sb.tile([DHEAD, LCH], F32, tag="os", bufs=2)
        nc.vector.tensor_copy(os_sb, op[0:DHEAD, :])
        dn = sb.tile([DHEAD, LCH], F32, tag="dn", bufs=2)
        nc.vector.tensor_copy(dn, op[DHEAD:P, :])
        rs = sb.tile([DHEAD, LCH], F32, tag="rs", bufs=2)
        nc.vector.reciprocal_approx_fast(rs, dn)
        nc.gpsimd.tensor_mul(
            o_norm[ds(64 * h, 64), p, ds(lc, LCH)], os_sb, rs)

    def emit_outproj(lc, nlts=LCH // P):
        for lt in range(lc // P, lc // P + nlts):
            for mc in range(NMC):
                yp = op_tile(name="yp")
                for pp in range(NPAIR):
                    nc.tensor.matmul(
                        yp, lhsT=o_norm[:, pp, ds(lt * P, P)],
                        rhs=wd_sb[:, pp, ds(mc * MCH, MCH)],
                        start=(pp == 0), stop=(pp == NPAIR - 1))
                ys = sb.tile([P, MCH], F32, tag="ys", bufs=2)
                nc.vector.tensor_copy(ys, yp)
                nc.sync.dma_start(
                    y_d[ds(lt * P, P), ds(mc * MCH, MCH)], ys)

    # ---- window -1: scores(0,0)+exp, V, then boundary projections ----
    emit_scores(0, 0, range(NLT))
    emit_v()
    qkv_proj(qT[0], 0, 0, range(ECH // LCH, 2 * ECH // LCH))   # Q(p0, ec1)
    qkv_proj(kT[1], 1, 1, range(2))                            # K(p1) ch 0-1

    # ---- windows over chunks: attend(k) ∥ scores+exp(k+1) per l-tile ----
    for ci, (p, ec) in enumerate(CHUNKS):
        nxt = CHUNKS[ci + 1] if ci + 1 < len(CHUNKS) else None
        ops = [[op_tile(name=f"at{ci}{sub}{h}") for h in range(2)]
               for sub in range(ECH // LCH)]
        for lt in range(NLT):
            for sub in range(ECH // LCH):
                for h in range(2):
                    nc.tensor.matmul(
                        ops[sub][h], lhsT=vt[:, lt, 2 * p + h],
                        rhs=pt_tiles[(p, ec)][h][lt][:, ds(sub * LCH, LCH)],
                        start=(lt == 0), stop=(lt == NLT - 1))
            if nxt is not None:
                emit_scores(nxt[0], nxt[1], [lt])
        # boundary: normalize burst + projections for upcoming chunks
        for sub in range(ECH // LCH):
            for h in range(2):
                norm_one(p, ec, sub, h, ops[sub][h])
        if ci == 0:
            qkv_proj(kT[1], 1, 1, range(2, 4))                 # K(p1) ch 2-3
            qkv_proj(qT[1], 1, 0, range(ECH // LCH))           # Q(p1, ec0)
        elif ci == 1:
            qkv_proj(qT[1], 1, 0, range(ECH // LCH, 2 * ECH // LCH))
        if p == NPAIR - 1:
            emit_outproj(ec * ECH, nlts=ECH // P)


_NC_CACHE = {}


def _get_nc():
    if "nc" not in _NC_CACHE:
        _NC_CACHE["nc"] = build_nc()
    return _NC_CACHE["nc"]


def shard_inputs(x, Wq, bq, Wk, bk, Wv, bv, Wd, bd):
    """Build the 8 per-core input maps."""
    in_maps = []
    for c in range(NCORES):
        b = c // (NCORES // B)
        h0 = (c % (NCORES // B)) * H_PER_CORE
        hs = slice(h0, h0 + H_PER_CORE)
        in_maps.append({
            "x": np.ascontiguousarray(np.asarray(x[b], np.float32)),
            "wq": np.ascontiguousarray(np.asarray(Wq[:, hs, :], np.float32).reshape(DMODEL, -1)),
            "wk": np.ascontiguousarray(np.asarray(Wk[:, hs, :], np.float32).reshape(DMODEL, -1)),
            "wv": np.ascontiguousarray(np.asarray(Wv[:, hs, :], np.float32).reshape(DMODEL, -1)),
            "wd": np.ascontiguousarray(np.asarray(Wd[hs], np.float32).reshape(-1, DMODEL)),
            "bq": np.ascontiguousarray(np.asarray(bq[hs], np.float32).reshape(-1)),
            "bk": np.ascontiguousarray(np.asarray(bk[hs], np.float32).reshape(-1)),
            "bv": np.ascontiguousarray(np.asarray(bv[hs], np.float32).reshape(-1)),
        })
    return in_maps


def gather_outputs(results, bd):
    """Sum partial outputs per batch and add bd."""
    out = np.zeros((B, L, DMODEL), np.float32)
    per_b = NCORES // B
    for c, res in enumerate(results):
        out[c // per_b] += res["y"]
    out += np.asarray(bd, np.float32)[None, None, :]
    return out


def kernel(x, Wq, bq, Wk, bk, Wv, bv, Wd, bd, _trace=False):
    nc = _get_nc()
    in_maps = shard_inputs(x, Wq, bq, Wk, bk, Wv, bv, Wd, bd)
    res = run_bass_kernel_spmd(nc, in_maps, list(range(NCORES)), trace=_trace)
    out = gather_outputs(res.results, bd)
    if _trace:
        kernel.last_results = res
    return out


# revision 17
# speedup vs baseline: 1.2512x; 1.0490x over previous
"""Trainium2 Bass kernel: multi-head attention (dense transformer block).

Computation (per batch b):
    Q = x @ Wq + bq ; K = x @ Wk + bk ; V = x @ Wv + bv        (per head)
    P = exp((Q @ K^T) / sqrt(Dh))                   (no max-subtraction needed:
                                                     scores are O(1) by construction)
    out = sum_h (P @ V / rowsum(P)) @ Wd[h] + bd

Sharding (data + tensor parallel): 8 cores; core c handles batch b = c // 4
and the 4 heads starting at 4*(c % 4). Each core computes a partial [L, D]
output; the host sums the 4 partials per batch and adds bd.

v4 design notes (per-core):
  - The kernel is a PE/ACT "ridge": matmul stream floor ~143us, exp on the
    scalar(ACT) engine ~146us irreducible. ScalarE runs NOTHING but the 128
    exp instructions; drains/casts/copies live on DVE or GpSimd.
  - Four (pair, ec) chunks are software-pipelined per l-tile: window k runs
    attend(chunk k) and scores+exp(chunk k+1) interleaved at l-tile
    granularity, so the ACT engine never waits at chunk boundaries. The
    attend uses 4 PSUM accumulators (one per (sub, h) stream), freeing a
    full 4-bank double-buffered score slot; a "normalize burst" at each
    window boundary drains all four, while the PE runs the next chunk's
    K/Q projections (or the out-projection) in the gap.
  - x DMAs stream on the sync HWDGE queue; weights ride the scalar HWDGE
    queue in parallel, so the first K-projection starts ~5us in.
  - Per pair: K^T/Q^T bf16 with dual-head 64-row stacking; scores
    S^T = K^T.T @ Q^T land softmax-axis-on-partitions; the two heads run
    on independent 64-row PE tiles concurrently. exp on ScalarE
    (scale=1/8 fused), 1024 wide, PSUM->SBUF bf16.
  - V for BOTH pairs in one pass ([l',d] layout, N=256 matmuls); its DVE
    drain applies bv and interleaves the ones-columns of the denominator
    trick.
  - attend: O^T = [V_h | ones].T @ P^T; PSUM rows 64..127 = rowsum(P),
    broadcast for free. Normalize: DVE drains + reciprocal_approx_fast
    (fed from a partition-0 tile; the custom op mishandles base!=0) +
    GpSimd multiply.
All matmul operands are bf16 (fp32 accumulation in PSUM).
"""

import os
import sys
from contextlib import ExitStack

import numpy as np

for _p in ("/opt/trn_rl_repo", "/root/.axon_site/_ro/trn_rl_repo"):
    if os.path.isdir(_p) and _p not in sys.path:
        sys.path.append(_p)

import concourse.bass as bass
import concourse.tile as tile
from concourse import bacc, mybir
from concourse.bass import ds, ts
from concourse.bass_utils import run_bass_kernel_spmd
from concourse.masks import make_identity

F32 = mybir.dt.float32
BF16 = mybir.dt.bfloat16

# Problem sizes (hardcoded per contract).
DMODEL, HEADS, DHEAD = 1024, 16, 64
B, L = 2, 2048
NCORES = 8
H_PER_CORE = B * HEADS // NCORES          # 4 heads per core
NPAIR = H_PER_CORE // 2                   # head pairs per core
P = 128                                   # partitions
KT = DMODEL // P                          # 8 k-tiles over dmodel
NLT = L // P                              # 16 l-tiles
LCH = 512                                 # matmul free-dim chunk (one psum bank)
ECH = 1024                                # exp chunk (2 psum banks)
NEC = L // ECH                            # 2 exp chunks
MCH = 512                                 # m-chunk for out-proj
NMC = DMODEL // MCH
PT_BUFS = 34                              # score-tile ring (SBUF budget bound)
CHUNKS = [(p, ec) for p in range(NPAIR) for ec in range(NEC)]


def build_nc():
    """Build the SPMD Bass program for one core."""
    nc = bacc.Bacc("TRN2", target_bir_lowering=False, debug=False,
                   num_devices=NCORES)

    x_d = nc.dram_tensor("x", [L, DMODEL], F32, kind="ExternalInput").ap()
    wq_d = nc.dram_tensor("wq", [DMODEL, H_PER_CORE * DHEAD], F32, kind="ExternalInput").ap()
    wk_d = nc.dram_tensor("wk", [DMODEL, H_PER_CORE * DHEAD], F32, kind="ExternalInput").ap()
    wv_d = nc.dram_tensor("wv", [DMODEL, H_PER_CORE * DHEAD], F32, kind="ExternalInput").ap()
    wd_d = nc.dram_tensor("wd", [H_PER_CORE * DHEAD, DMODEL], F32, kind="ExternalInput").ap()
    bq_d = nc.dram_tensor("bq", [H_PER_CORE * DHEAD], F32, kind="ExternalInput").ap()
    bk_d = nc.dram_tensor("bk", [H_PER_CORE * DHEAD], F32, kind="ExternalInput").ap()
    bv_d = nc.dram_tensor("bv", [H_PER_CORE * DHEAD], F32, kind="ExternalInput").ap()
    y_d = nc.dram_tensor("y", [L, DMODEL], F32, kind="ExternalOutput").ap()

    with ExitStack() as ctx:
        tc = ctx.enter_context(tile.TileContext(nc))
        _body(nc, tc, ctx, x_d, wq_d, wk_d, wv_d, wd_d, bq_d, bk_d, bv_d, y_d)
    nc.compile()
    return nc


def _body(nc, tc, ctx, x_d, wq_d, wk_d, wv_d, wd_d, bq_d, bk_d, bv_d, y_d):
    const = ctx.enter_context(tc.tile_pool(name="const", bufs=1))
    sb = ctx.enter_context(tc.tile_pool(name="sb", bufs=1))
    psum = ctx.enter_context(tc.tile_pool(name="psum", bufs=1, space="PSUM"))

    ident = const.tile([P, P], BF16)
    make_identity(nc, ident)

    # biases (tiny, SWDGE): [P, {q,k}, pair] per-partition scalars
    bias_sb = const.tile([P, 2, NPAIR], F32)
    for i, b_d in enumerate((bq_d, bk_d)):
        for p in range(NPAIR):
            nc.gpsimd.dma_start(bias_sb[:, i, p:p + 1],
                                b_d.rearrange("(a p) -> a p", p=P)[p:p + 1, :]
                                .rearrange("a p -> p a"))
    bv_rep = const.tile([P, NPAIR * P], F32)
    nc.gpsimd.dma_start(bv_rep, bass.AP(tensor=bv_d.tensor, offset=0,
                                        ap=[[0, P], [1, NPAIR * P]]))

    # weights: fp32 on the SCALAR HWDGE queue (parallel to x on sync), DVE cast
    w_sb = const.tile([P, NPAIR, 3, KT, P], BF16)   # [k, pair, {q,k,v}, kt, cols]
    wd_sb = const.tile([P, NPAIR, DMODEL], BF16)

    def w_load(i, w_d):
        # full-matrix load: 1KB-contiguous rows, both pairs at once
        ws = sb.tile([P, KT, 2 * P], F32, tag="wstage", bufs=1)
        nc.scalar.dma_start(ws, w_d.rearrange("(kt k) m -> k kt m", k=P))
        for p in range(NPAIR):
            nc.vector.tensor_copy(w_sb[:, p, i], ws[:, :, ds(p * P, P)])

    w_load(1, wk_d)   # K first: first projection to run
    w_load(0, wq_d)
    w_load(2, wv_d)
    for pp in range(NPAIR):
        wds = sb.tile([P, DMODEL], F32, tag="wds", bufs=2, name="wds")
        nc.scalar.dma_start(
            wds, wd_d.rearrange("(pp k) m -> k pp m", k=P)[:, pp])
        nc.vector.tensor_copy(wd_sb[:, pp], wds)

    # PSUM: "sp" 2x[P,1024]f32 (4 banks) for scores; "op" 4x[P,512]f32
    # (4 banks) for the attend accumulators, timeshared at boundaries by
    # projections / transposes / out-proj.
    def op_tile(shape=None, dt=F32, name="opx"):
        return psum.tile(shape or [P, LCH], dt, tag="op", bufs=4, name=name)

    o_norm = sb.tile([P, NPAIR, L], BF16)
    vt = sb.tile([P, NLT, 2 * NPAIR, 2, DHEAD], BF16)
    xt = sb.tile([P, KT, L], BF16)

    def qkv_proj(dst, p, i, lcs):
        for lc in lcs:
            ps = op_tile(name="qkvps")
            for kt in range(KT):
                nc.tensor.matmul(
                    ps, lhsT=w_sb[:, p, i, kt],
                    rhs=xt[:, kt, ds(lc * LCH, LCH)],
                    start=(kt == 0), stop=(kt == KT - 1))
            nc.vector.tensor_scalar_add(
                dst[:, ds(lc * LCH, LCH)], ps, bias_sb[:, i, p:p + 1])

    # ---- phase 0: x fp32 in (sync queue), DVE cast, PE transposes ----
    qT = {}
    kT = {}
    for p in range(NPAIR):
        qT[p] = sb.tile([P, L], BF16, tag="qkv0", bufs=NPAIR, name=f"qT{p}")
        kT[p] = sb.tile([P, L], BF16, tag="qkv1", bufs=NPAIR, name=f"kT{p}")

    for g in range(4):
        for lt in range(4 * g, 4 * g + 4):
            xs = sb.tile([P, DMODEL], F32, tag="xs", bufs=3)
            nc.sync.dma_start(xs, x_d[ds(lt * P, P), :])
            xb = sb.tile([P, DMODEL], BF16, tag="xb", bufs=3)
            nc.vector.tensor_copy(xb, xs)
            tp = op_tile([P, KT, P], BF16, name="tp")
            for kt in range(KT):
                nc.tensor.transpose(tp[:, kt], xb[:, ds(kt * P, P)], ident)
            nc.vector.tensor_copy(xt[:, :, ds(lt * P, P)], tp)
        qkv_proj(kT[0], 0, 1, [g])
        if g < 2:
            qkv_proj(qT[0], 0, 0, [g])

    pt_tiles = {key: [[None] * NLT, [None] * NLT] for key in CHUNKS}

    def emit_scores(p, ec, lts):
        for lt in lts:
            for h in range(2):
                sp = psum.tile([P, ECH], F32, tag="sp", bufs=2, name="sp")
                for sub in range(ECH // LCH):
                    nc.tensor.matmul(
                        sp[:, ds(sub * LCH, LCH)],
                        lhsT=kT[p][ds(64 * h, 64), ds(lt * P, P)],
                        rhs=qT[p][ds(64 * h, 64),
                                  ds(ec * ECH + sub * LCH, LCH)],
                        start=True, stop=True)
                pt = sb.tile([P, ECH], BF16, tag="pt", bufs=PT_BUFS)
                nc.scalar.activation(
                    pt, sp, func=mybir.ActivationFunctionType.Exp,
                    scale=1.0 / np.sqrt(DHEAD))
                pt_tiles[(p, ec)][h][lt] = pt

    def emit_v():
        nc.vector.memset(vt[:, :, :, 1, :], 1.0)
        for lt in range(NLT):
            vp = op_tile([P, 2 * NPAIR * DHEAD], name="vp")
            for kt in range(KT):
                nc.tensor.matmul(
                    vp, lhsT=xt[:, kt, ds(lt * P, P)],
                    rhs=w_sb[:, :, 2, kt, :],
                    start=(kt == 0), stop=(kt == KT - 1))
            nc.vector.tensor_add(
                vt[:, lt, :, 0, :],
                vp.rearrange("a (g d) -> a g d", d=DHEAD),
                bv_rep.rearrange("a (g d) -> a g d", d=DHEAD))

    def norm_one(p, ec, sub, h, op):
        lc = ec * ECH + sub * LCH
        os_sb = sb.tile([DHEAD, LCH], F32, tag="os", bufs=2)
        nc.vector.tensor_copy(os_sb, op[0:DHEAD, :])
        dn = sb.tile([DHEAD, LCH], F32, tag="dn", bufs=2)
        nc.vector.tensor_copy(dn, op[DHEAD:P, :])
        rs = sb.tile([DHEAD, LCH], F32, tag="rs", bufs=2)
        nc.vector.reciprocal_approx_fast(rs, dn)
        nc.gpsimd.tensor_mul(
            o_norm[ds(64 * h, 64), p, ds(lc, LCH)], os_sb, rs)

    def emit_outproj(lc, nlts=LCH // P):
        for lt in range(lc // P, lc // P + nlts):
            for mc in range(NMC):
                yp = op_tile(name="yp")
                for pp in range(NPAIR):
                    nc.tensor.matmul(
                        yp, lhsT=o_norm[:, pp, ds(lt * P, P)],
                        rhs=wd_sb[:, pp, ds(mc * MCH, MCH)],
                        start=(pp == 0), stop=(pp == NPAIR - 1))
                ys = sb.tile([P, MCH], F32, tag="ys", bufs=2)
                nc.vector.tensor_copy(ys, yp)
                nc.sync.dma_start(
                    y_d[ds(lt * P, P), ds(mc * MCH, MCH)], ys)

    # ---- window -1: scores(0,0)+exp, V, then boundary projections ----
    emit_scores(0, 0, range(NLT))
    emit_v()
    qkv_proj(qT[0], 0, 0, range(ECH // LCH, 2 * ECH // LCH))   # Q(p0, ec1)
    qkv_proj(kT[1], 1, 1, range(2))                            # K(p1) ch 0-1

    # ---- windows over chunks 0-2: attend(k) ∥ scores+exp(k+1) per l-tile ----
    for ci, (p, ec) in enumerate(CHUNKS[:-1]):
        nxt = CHUNKS[ci + 1]
        ops = [[op_tile(name=f"at{ci}{sub}{h}") for h in range(2)]
               for sub in range(ECH // LCH)]
        for lt in range(NLT):
            for sub in range(ECH // LCH):
                for h in range(2):
                    nc.tensor.matmul(
                        ops[sub][h], lhsT=vt[:, lt, 2 * p + h],
                        rhs=pt_tiles[(p, ec)][h][lt][:, ds(sub * LCH, LCH)],
                        start=(lt == 0), stop=(lt == NLT - 1))
            emit_scores(nxt[0], nxt[1], [lt])
        # boundary: normalize burst + projections for upcoming chunks
        for sub in range(ECH // LCH):
            for h in range(2):
                norm_one(p, ec, sub, h, ops[sub][h])
        if ci == 0:
            qkv_proj(kT[1], 1, 1, range(2, 4))                 # K(p1) ch 2-3
            qkv_proj(qT[1], 1, 0, range(ECH // LCH))           # Q(p1, ec0)
        elif ci == 1:
            qkv_proj(qT[1], 1, 0, range(ECH // LCH, 2 * ECH // LCH))

    # ---- window 3 (chunk (1,1)): two half-sweeps (sub at a time) with the
    # out-projection of already-normalized l-ranges riding along per l-tile,
    # so the post-exp tail is just the last half-chunk's drain ----
    p, ec = CHUNKS[-1]
    for sub in range(ECH // LCH):
        ops1 = [op_tile(name=f"fin{sub}{h}") for h in range(2)]
        # l-tiles whose o_norm is already complete: ec0's 8, then ec1-sub0's 4
        ready = list(range(8)) if sub == 0 else list(range(8, 12))
        for lt in range(NLT):
            for h in range(2):
                nc.tensor.matmul(
                    ops1[h], lhsT=vt[:, lt, 2 * p + h],
                    rhs=pt_tiles[(p, ec)][h][lt][:, ds(sub * LCH, LCH)],
                    start=(lt == 0), stop=(lt == NLT - 1))
            if sub == 0:
                if lt % 2 == 0:
                    emit_outproj(ready[lt // 2] * P, nlts=1)
            elif lt < 2 * len(ready):
                if lt % 2 == 0:
                    emit_outproj(ready[lt // 2] * P, nlts=1)
        for h in range(2):
            norm_one(p, ec, sub, h, ops1[h])
    emit_outproj(ECH + LCH, nlts=LCH // P)


_NC_CACHE = {}


def _get_nc():
    if "nc" not in _NC_CACHE:
        _NC_CACHE["nc"] = build_nc()
    return _NC_CACHE["nc"]


def shard_inputs(x, Wq, bq, Wk, bk, Wv, bv, Wd, bd):
    """Build the 8 per-core input maps."""
    in_maps = []
    for c in range(NCORES):
        b = c // (NCORES // B)
        h0 = (c % (NCORES // B)) * H_PER_CORE
        hs = slice(h0, h0 + H_PER_CORE)
        in_maps.append({
            "x": np.ascontiguousarray(np.asarray(x[b], np.float32)),
            "wq": np.ascontiguousarray(np.asarray(Wq[:, hs, :], np.float32).reshape(DMODEL, -1)),
            "wk": np.ascontiguousarray(np.asarray(Wk[:, hs, :], np.float32).reshape(DMODEL, -1)),
            "wv": np.ascontiguousarray(np.asarray(Wv[:, hs, :], np.float32).reshape(DMODEL, -1)),
            "wd": np.ascontiguousarray(np.asarray(Wd[hs], np.float32).reshape(-1, DMODEL)),
            "bq": np.ascontiguousarray(np.asarray(bq[hs], np.float32).reshape(-1)),
            "bk": np.ascontiguousarray(np.asarray(bk[hs], np.float32).reshape(-1)),
            "bv": np.ascontiguousarray(np.asarray(bv[hs], np.float32).reshape(-1)),
        })
    return in_maps


def gather_outputs(results, bd):
    """Sum partial outputs per batch and add bd."""
    out = np.zeros((B, L, DMODEL), np.float32)
    per_b = NCORES // B
    for c, res in enumerate(results):
        out[c // per_b] += res["y"]
    out += np.asarray(bd, np.float32)[None, None, :]
    return out


def kernel(x, Wq, bq, Wk, bk, Wv, bv, Wd, bd, _trace=False):
    nc = _get_nc()
    in_maps = shard_inputs(x, Wq, bq, Wk, bk, Wv, bv, Wd, bd)
    res = run_bass_kernel_spmd(nc, in_maps, list(range(NCORES)), trace=_trace)
    out = gather_outputs(res.results, bd)
    if _trace:
        kernel.last_results = res
    return out
